# revision 12
# baseline (speedup 1.0000x reference)
"""Trainium2 Bass kernel for the BSDE solver (nn_BSDESolver).

Math (per path, M=50 steps, a = 1+R*DT):
  S_{i+1} = S_i * g_i,  g_i = a + SIGMA*dw_i          (z-independent GBM)
  Y_M = a^M Y0 + sum_i gam_i * z_i * x_i * dw_i,  gam_i = SIGMA*S0*a^(49-i)
  x_i = S_i/S0 = exp(u_i),  u_i = sum_{j<i} ln(a + SIGMA*dw_j)

z_i = MLP(x_i, t_i) is fitted per step as z ~ b0/x + b1 + b2 x^2 so that
z*x*dw = b0*dw + b1*w + b2*P2 with w = x*dw, P2 = x^3*dw -- three
contraction matmuls against per-step coefficient columns.

Layout (the big win vs the 100/128-row predecessor): each SBUF column
packs FIVE 25-step path-halves -> 125 of 128 partitions carry data, and
every engine on TRN2 charges by free-dim columns only.  A path's steps
0-24 live in slab "A", steps 25-49 in slab "B" at the same column; B's
prefix adds the A-half total via a second accumulated matmul (TRIZBA =
group-summed ones).  Step 0 has x=1 exactly, so its term is folded as a
constant into CB-A's dw coefficient and TRIZ column 25g+0 is zero.
S_50 = S0*(a*x_49 + SIGMA*w_49) is picked up by one extra matmul on the
x tile (XPB) plus an S0*SIGMA coefficient in CB-B1.

Contraction matmuls have only 10 useful output rows, so the four
512-column blocks of a quad run CONCURRENTLY in the four 32-column
strips of the PE array via tile_position=(0,32jj) -- ~4x on the
contraction part of the PE timeline.

GpSimd is deliberately unused: its tensor ops are Q7 software loops
measured ~20x slower than the cost model claims on this hardware.
Data parallel over batch across 8 cores; dw ships as bf16.
"""
import numpy as np

import concourse.mybir as mybir
import concourse.tile as tile
import concourse.bacc as bacc
from concourse import bass_utils

F32 = mybir.dt.float32
BF16 = mybir.dt.bfloat16
AF = mybir.ActivationFunctionType
ALU = mybir.AluOpType

S0, R, SIGMA, T = 100.0, 0.05, 0.2, 1.0
M = 50
DT = T / M
RDT = R * DT
A = 1.0 + RDT
NCORES = 8
B_FULL = 1048576
B_CORE = B_FULL // NCORES          # 131072 paths
G = 5                              # path-halves packed per column
SH = 25                            # steps per half
NROW = G * SH                      # 125 used partitions
NCOL = 26624                       # ceil(B_CORE/G) padded to 52*512
NQ = NCOL // 2048                  # 13 column quads (each has an A+B slab)
LQ = 3                             # contraction lookahead (quad-slabs)


def _zeta_np(x, t, W1, b1, W2, b2, W3, b3):
    h = np.tanh(np.stack([x, np.full_like(x, t)], 1) @ W1 + b1)
    h = np.tanh(h @ W2 + b2)
    return 1.0 / (1.0 + np.exp(-(h @ W3 + b3)))[:, 0]


def _fit_beta(W1, b1, W2, b2, W3, b3, ts):
    """Per-step fit of z_s(x) in basis {x^-1, 1, x^2}, x = S/S0, on a
    Chebyshev grid of u = log x covering +-7.5 sigma of the step's
    log-price distribution (IRLS sharpens LS toward minimax)."""
    sdt = SIGMA * np.sqrt(DT)
    beta = np.zeros((M, 3), np.float64)
    th = np.linspace(0.0, np.pi, 801)
    grid01 = 0.5 * (1.0 - np.cos(th))
    for s in range(M):
        std = sdt * np.sqrt(max(s, 1))
        drift = s * (RDT - 0.5 * SIGMA * SIGMA * DT)
        half = max(7.5 * std, 0.02)
        u = (drift - half) + 2.0 * half * grid01
        x = np.exp(u)
        z = _zeta_np(x, ts[s], W1, b1, W2, b2, W3, b3)
        Am = np.stack([1.0 / x, np.ones_like(x), x * x], 1)
        wgt = np.ones_like(z)
        for _ in range(6):
            c, *_ = np.linalg.lstsq(Am * wgt[:, None], z * wgt, rcond=None)
            r = np.abs(Am @ c - z)
            wgt = np.sqrt(wgt * np.maximum(r / max(r.max(), 1e-12), 1e-3))
        beta[s] = c
    return beta


def _build_consts(W1, b1, W2, b2, W3, b3, ts):
    import ml_dtypes
    beta = _fit_beta(W1, b1, W2, b2, W3, b3, ts)
    gam = SIGMA * S0 * A ** (49 - np.arange(M, dtype=np.float64))

    # prefix matrices: column 25g+i sums rows 25g+j (j<i) of its own slab;
    # TRIZBA adds the whole A-half of the group into every B prefix.
    TRIZA = np.zeros((NROW, 128), np.float32)
    TRIZB = np.zeros((NROW, 128), np.float32)
    TRIZBA = np.zeros((NROW, 128), np.float32)
    for g in range(G):
        for i in range(SH):
            TRIZA[25 * g:25 * g + i, 25 * g + i] = 1.0
            TRIZB[25 * g:25 * g + i, 25 * g + i] = 1.0
            TRIZBA[25 * g:25 * g + 25, 25 * g + i] = 1.0

    # contraction: out row g = Y-partial of group g, row 5+g = S50
    CBA = np.zeros((3, NROW, 32), np.float32)
    CBB = np.zeros((3, NROW, 32), np.float32)
    XPB = np.zeros((NROW, 32), np.float32)
    z1 = _zeta_np(np.ones(1), ts[0], W1, b1, W2, b2, W3, b3)[0]
    for g in range(G):
        for i in range(SH):
            if i == 0:
                # x_0 == 1 exactly: fold gam_0 * z(1,t_0) into the dw tile
                CBA[0, 25 * g + 0, g] = gam[0] * z1
            else:
                for k in range(3):
                    CBA[k, 25 * g + i, g] = gam[i] * beta[i, k]
            for k in range(3):
                CBB[k, 25 * g + i, g] = gam[25 + i] * beta[25 + i, k]
        CBB[1, 25 * g + 24, 5 + g] = S0 * SIGMA   # S50 = S0*(a*x49 + s*w49)
        XPB[25 * g + 24, 5 + g] = S0 * A

    c = {"TRIZA": TRIZA, "TRIZB": TRIZB, "TRIZBA": TRIZBA,
         "CBA0": CBA[0], "CBA1": CBA[1], "CBA2": CBA[2],
         "CBB0": CBB[0], "CBB1": CBB[1], "CBB2": CBB[2], "XPB": XPB}
    return {k: v.astype(ml_dtypes.bfloat16) for k, v in c.items()}


CDEFS = [("TRIZA", [NROW, 128]), ("TRIZB", [NROW, 128]),
         ("TRIZBA", [NROW, 128]),
         ("CBA0", [NROW, 32]), ("CBA1", [NROW, 32]), ("CBA2", [NROW, 32]),
         ("CBB0", [NROW, 32]), ("CBB1", [NROW, 32]), ("CBB2", [NROW, 32]),
         ("XPB", [NROW, 32])]


def _build_kernel(num_devices, nreps=1):
    nc = bacc.Bacc("TRN2", debug=False, num_devices=num_devices,
                   target_bir_lowering=False)

    # Ln+Exp live together in "natural_log_exp_and_others"; pin that one
    # table for the whole program to avoid dynamic table reloads.
    from concourse.hw_specs import get_activation_tables
    import concourse.bacc as _bacc_mod

    def _pinned_act_table_loads():
        tables = []
        for name, funcs in get_activation_tables(nc.m.arch).items():
            tables.append((name, funcs if name == "natural_log_exp_and_others"
                           else set()))
        _bacc_mod._bass_rust.insert_act_table_loads(nc, tables)

    nc.insert_act_table_loads = _pinned_act_table_loads
    tc = tile.TileContext(nc)

    dwTA = nc.dram_tensor("dwTA", [128, NCOL], BF16, kind="ExternalInput")
    dwTB = nc.dram_tensor("dwTB", [128, NCOL], BF16, kind="ExternalInput")
    cins = {n: nc.dram_tensor(n, s, BF16, kind="ExternalInput")
            for n, s in CDEFS}
    lbin = nc.dram_tensor("LB", [128, 1], F32, kind="ExternalInput")
    Zout = nc.dram_tensor("Zout", [NQ * 128, 512], F32, kind="ExternalOutput")

    NT = 2 * NQ                     # quad-slabs per pass (A/B interleaved)

    with tc:
        with tc.tile_pool(name="consts", bufs=1) as cpool, \
             tc.tile_pool(name="inp", bufs=4) as ipool, \
             tc.tile_pool(name="lgp", bufs=3) as lpool, \
             tc.tile_pool(name="xwp", bufs=LQ + 2) as xpool, \
             tc.tile_pool(name="p1p", bufs=2) as ppool, \
             tc.tile_pool(name="stg", bufs=2) as spool, \
             tc.tile_pool(name="ps_pref", bufs=3, space="PSUM") as p_pref, \
             tc.tile_pool(name="ps_y", bufs=2, space="PSUM") as p_y:

            C = {}
            for n, s in CDEFS:
                C[n] = cpool.tile(s, BF16, name=f"c_{n}", tag=f"c_{n}")
                nc.sync.dma_start(C[n][:], cins[n].ap())
            LB = cpool.tile([128, 1], F32, name="c_LB", tag="c_LB")
            nc.sync.dma_start(LB[:], lbin.ap())

            dwt = {}
            lg = {}
            xq = {}
            wq = {}
            p2q = {}
            ypt = {}

            def dma_in(j):
                dwt[j] = ipool.tile([128, 4096], BF16, name="dw", tag="dw")
                cols = slice(2048 * j, 2048 * (j + 1))
                nc.sync.dma_start(dwt[j][:, 0:2048], dwTA.ap()[:, cols])
                nc.sync.dma_start(dwt[j][:, 2048:4096], dwTB.ap()[:, cols])

            def ln_op(j):
                lg[j] = lpool.tile([128, 4096], BF16, name="lg", tag="lg")
                nc.scalar.activation(lg[j][:], dwt[j][:], AF.Ln,
                                     bias=LB[:], scale=SIGMA)

            def front(t):
                j, ph = t // 2, t % 2
                so = 2048 * ph
                TRZ = C["TRIZB"] if ph else C["TRIZA"]
                x = xpool.tile([128, 2048], BF16, name="x", tag="x")
                for h in range(2):
                    pref = p_pref.tile([128, 1024], F32, name="pref",
                                       tag="pref")
                    for k2 in range(2):
                        c0 = 1024 * h + 512 * k2
                        oc = slice(512 * k2, 512 * (k2 + 1))
                        nc.tensor.matmul(pref[:, oc], TRZ[:],
                                         lg[j][0:NROW, so + c0:so + c0 + 512],
                                         start=True, stop=(ph == 0))
                        if ph == 1:
                            nc.tensor.matmul(pref[:, oc], C["TRIZBA"][:],
                                             lg[j][0:NROW, c0:c0 + 512],
                                             start=False, stop=True)
                    nc.scalar.activation(x[:, 1024 * h:1024 * (h + 1)],
                                         pref[:], AF.Exp)
                if ph == 0 and j + 1 < NQ:
                    ln_op(j + 1)
                dws = dwt[j][:, so:so + 2048]
                w = xpool.tile([128, 2048], BF16, name="w", tag="w")
                nc.vector.tensor_tensor(w[:], x[:], dws, op=ALU.mult)
                p1 = ppool.tile([128, 2048], BF16, name="p1", tag="p1")
                nc.vector.tensor_tensor(p1[:], w[:], x[:], op=ALU.mult)
                p2 = xpool.tile([128, 2048], BF16, name="p2", tag="p2")
                nc.vector.tensor_tensor(p2[:], p1[:], x[:], op=ALU.mult)
                xq[t], wq[t], p2q[t] = x, w, p2

            def contract(t):
                j, ph = t // 2, t % 2
                so = 2048 * ph
                if ph == 0:
                    ypt[j] = p_y.tile([128, 512], F32, name="yp", tag="yp")
                yp = ypt[j]
                cb = ("CBB0", "CBB1", "CBB2") if ph else \
                     ("CBA0", "CBA1", "CBA2")
                for jj in range(4):
                    lc = slice(512 * jj, 512 * (jj + 1))
                    ys = yp[32 * jj:32 * (jj + 1), :]
                    tp = (0, 32 * jj)
                    nc.tensor.matmul(ys, C[cb[0]][:],
                                     dwt[j][0:NROW, so + 512 * jj:
                                            so + 512 * (jj + 1)],
                                     start=(ph == 0), stop=False,
                                     tile_position=tp)
                    nc.tensor.matmul(ys, C[cb[1]][:], wq[t][0:NROW, lc],
                                     start=False, stop=False,
                                     tile_position=tp)
                    nc.tensor.matmul(ys, C[cb[2]][:], p2q[t][0:NROW, lc],
                                     start=False, stop=False,
                                     tile_position=tp)
                    if ph == 1:
                        nc.tensor.matmul(ys, C["XPB"][:],
                                         xq[t][0:NROW, lc],
                                         start=False, stop=True,
                                         tile_position=tp)
                del xq[t], wq[t], p2q[t]
                if ph == 1:
                    del dwt[j], lg[j]
                    stg = spool.tile([128, 512], F32, name="st", tag="st")
                    nc.vector.tensor_copy(stg[:], yp[:])
                    nc.sync.dma_start(Zout.ap()[128 * j:128 * (j + 1), :],
                                      stg[:])

            for rep in range(nreps):
                dma_in(0)
                dma_in(1)
                ln_op(0)
                for t in range(NT + LQ):
                    if t < NT:
                        j, ph = t // 2, t % 2
                        if ph == 0 and j + 2 < NQ:
                            dma_in(j + 2)
                        front(t)
                    if t >= LQ:
                        contract(t - LQ)

    nc.compile()
    return nc


_CACHE = {}
_LAST_IN_MAPS = None


def kernel(dw, t_grid, W1, b1, W2, b2, W3, b3, Y0):
    import ml_dtypes
    dw = np.asarray(dw, np.float32)
    t_grid = np.asarray(t_grid, np.float32)
    B = dw.shape[0]
    assert B == B_FULL and dw.shape[1] == M
    a50y0 = np.float32(A ** M * np.float32(Y0))

    if "nc" not in _CACHE:
        _CACHE["nc"] = _build_kernel(NCORES)
    nc = _CACHE["nc"]

    consts = _build_consts(np.asarray(W1, np.float32),
                           np.asarray(b1, np.float32),
                           np.asarray(W2, np.float32),
                           np.asarray(b2, np.float32),
                           np.asarray(W3, np.float32),
                           np.asarray(b3, np.float32), t_grid[0])

    dwb = dw.astype(ml_dtypes.bfloat16)
    in_maps = []
    for ci in range(NCORES):
        blk = dwb[ci * B_CORE:(ci + 1) * B_CORE]
        pad = np.zeros((G * NCOL, M), ml_dtypes.bfloat16)
        pad[:B_CORE] = blk
        v = pad.reshape(G, NCOL, M)
        dwTA = np.zeros((128, NCOL), ml_dtypes.bfloat16)
        dwTB = np.zeros((128, NCOL), ml_dtypes.bfloat16)
        dwTA[0:NROW] = v[:, :, 0:SH].transpose(0, 2, 1).reshape(NROW, NCOL)
        dwTB[0:NROW] = v[:, :, SH:M].transpose(0, 2, 1).reshape(NROW, NCOL)
        mci = dict(consts)
        mci["dwTA"] = dwTA
        mci["dwTB"] = dwTB
        mci["LB"] = np.full((128, 1), A, np.float32)
        in_maps.append(mci)

    global _LAST_IN_MAPS
    _LAST_IN_MAPS = in_maps
    res = bass_utils.run_bass_kernel_spmd(nc, in_maps,
                                          core_ids=list(range(NCORES)))

    Y = np.empty((B_FULL,), np.float32)
    S = np.empty((B_FULL,), np.float32)
    for ci in range(NCORES):
        # Zout row 128j + 32jj + q: q in 0..4 -> Y of group q, 5..9 -> S50;
        # column c of that row -> path (q%5)*NCOL + 2048j + 512jj + c
        Z = res.results[ci]["Zout"].reshape(NQ, 4, 32, 512)
        Ymap = Z[:, :, 0:G, :].transpose(2, 0, 1, 3).reshape(G * NCOL)
        Smap = Z[:, :, G:2 * G, :].transpose(2, 0, 1, 3).reshape(G * NCOL)
        Y[ci * B_CORE:(ci + 1) * B_CORE] = Ymap[:B_CORE]
        S[ci * B_CORE:(ci + 1) * B_CORE] = Smap[:B_CORE]
    Y += a50y0
    return Y[:, None], S[:, None]


# revision 14
# speedup vs baseline: 1.0841x; 1.0841x over previous
"""Trainium2 Bass kernel for the BSDE solver (nn_BSDESolver).

Math (per path, M=50 steps, a = 1+R*DT):
  S_{i+1} = S_i * g_i,  g_i = a + SIGMA*dw_i          (z-independent GBM)
  Y_M = a^M Y0 + sum_i gam_i * z_i * x_i * dw_i,  gam_i = SIGMA*S0*a^(49-i)
  x_i = S_i/S0 = exp(u_i),  u_i = sum_{j<i} ln(a + SIGMA*dw_j)

z_i = MLP(x_i, t_i) is fitted per step as z ~ b0/x + b1 + b2 x^2 so that
z*x*dw = b0*dw + b1*w + b2*P2 with w = x*dw, P2 = x^3*dw -- three
contraction matmuls against per-step coefficient columns.

Layout (the big win vs the 100/128-row predecessor): each SBUF column
packs FIVE 25-step path-halves -> 125 of 128 partitions carry data, and
every engine on TRN2 charges by free-dim columns only.  A path's steps
0-24 live in slab "A", steps 25-49 in slab "B" at the same column; B's
prefix adds the A-half total via a second accumulated matmul (TRIZBA =
group-summed ones).  Step 0 has x=1 exactly, so its term is folded as a
constant into CB-A's dw coefficient and TRIZ column 25g+0 is zero.
S_50 = S0*(a*x_49 + SIGMA*w_49) is picked up by one extra matmul on the
x tile (XPB) plus an S0*SIGMA coefficient in CB-B1.

Contraction matmuls have only 10 useful output rows, so the four
512-column blocks of a quad run CONCURRENTLY in the four 32-column
strips of the PE array via tile_position=(0,32jj) -- ~4x on the
contraction part of the PE timeline.

GpSimd is deliberately unused: its tensor ops are Q7 software loops
measured ~20x slower than the cost model claims on this hardware.
Data parallel over batch across 8 cores; dw ships as bf16.
"""
import numpy as np

import concourse.mybir as mybir
import concourse.tile as tile
import concourse.bacc as bacc
from concourse import bass_utils

F32 = mybir.dt.float32
BF16 = mybir.dt.bfloat16
AF = mybir.ActivationFunctionType
ALU = mybir.AluOpType

S0, R, SIGMA, T = 100.0, 0.05, 0.2, 1.0
M = 50
DT = T / M
RDT = R * DT
A = 1.0 + RDT
NCORES = 8
B_FULL = 1048576
B_CORE = B_FULL // NCORES          # 131072 paths
G = 5                              # path-halves packed per column
SH = 25                            # steps per half
NROW = G * SH                      # 125 used partitions
NCOL = 26624                       # ceil(B_CORE/G) padded to 52*512
NQ = NCOL // 2048                  # 13 column quads (each has an A+B slab)
LQ = 3                             # contraction lookahead (quad-slabs)


def _zeta_np(x, t, W1, b1, W2, b2, W3, b3):
    h = np.tanh(np.stack([x, np.full_like(x, t)], 1) @ W1 + b1)
    h = np.tanh(h @ W2 + b2)
    return 1.0 / (1.0 + np.exp(-(h @ W3 + b3)))[:, 0]


def _fit_beta(W1, b1, W2, b2, W3, b3, ts):
    """Per-step fit of z_s(x) in basis {x^-1, 1, x^2}, x = S/S0, on a
    Chebyshev grid of u = log x covering +-7.5 sigma of the step's
    log-price distribution (IRLS sharpens LS toward minimax)."""
    sdt = SIGMA * np.sqrt(DT)
    beta = np.zeros((M, 3), np.float64)
    th = np.linspace(0.0, np.pi, 801)
    grid01 = 0.5 * (1.0 - np.cos(th))
    for s in range(M):
        std = sdt * np.sqrt(max(s, 1))
        drift = s * (RDT - 0.5 * SIGMA * SIGMA * DT)
        half = max(7.5 * std, 0.02)
        u = (drift - half) + 2.0 * half * grid01
        x = np.exp(u)
        z = _zeta_np(x, ts[s], W1, b1, W2, b2, W3, b3)
        Am = np.stack([1.0 / x, np.ones_like(x), x * x], 1)
        wgt = np.ones_like(z)
        for _ in range(6):
            c, *_ = np.linalg.lstsq(Am * wgt[:, None], z * wgt, rcond=None)
            r = np.abs(Am @ c - z)
            wgt = np.sqrt(wgt * np.maximum(r / max(r.max(), 1e-12), 1e-3))
        beta[s] = c
    return beta


def _build_consts(W1, b1, W2, b2, W3, b3, ts):
    import ml_dtypes
    beta = _fit_beta(W1, b1, W2, b2, W3, b3, ts)
    gam = SIGMA * S0 * A ** (49 - np.arange(M, dtype=np.float64))

    # prefix matrices: column 25g+i sums rows 25g+j (j<i) of its own slab;
    # TRIZBA adds the whole A-half of the group into every B prefix.
    TRIZA = np.zeros((NROW, 128), np.float32)
    TRIZB = np.zeros((NROW, 128), np.float32)
    TRIZBA = np.zeros((NROW, 128), np.float32)
    for g in range(G):
        for i in range(SH):
            TRIZA[25 * g:25 * g + i, 25 * g + i] = 1.0
            TRIZB[25 * g:25 * g + i, 25 * g + i] = 1.0
            TRIZBA[25 * g:25 * g + 25, 25 * g + i] = 1.0

    # contraction: out row g = Y-partial of group g, row 5+g = S50
    CBA = np.zeros((3, NROW, 32), np.float32)
    CBB = np.zeros((3, NROW, 32), np.float32)
    XPB = np.zeros((NROW, 32), np.float32)
    z1 = _zeta_np(np.ones(1), ts[0], W1, b1, W2, b2, W3, b3)[0]
    for g in range(G):
        for i in range(SH):
            if i == 0:
                # x_0 == 1 exactly: fold gam_0 * z(1,t_0) into the dw tile
                CBA[0, 25 * g + 0, g] = gam[0] * z1
            else:
                for k in range(3):
                    CBA[k, 25 * g + i, g] = gam[i] * beta[i, k]
            for k in range(3):
                CBB[k, 25 * g + i, g] = gam[25 + i] * beta[25 + i, k]
        CBB[1, 25 * g + 24, 5 + g] = S0 * SIGMA   # S50 = S0*(a*x49 + s*w49)
        XPB[25 * g + 24, 5 + g] = S0 * A

    c = {"TRIZA": TRIZA, "TRIZB": TRIZB, "TRIZBA": TRIZBA,
         "CBA0": CBA[0], "CBA1": CBA[1], "CBA2": CBA[2],
         "CBB0": CBB[0], "CBB1": CBB[1], "CBB2": CBB[2], "XPB": XPB}
    return {k: v.astype(ml_dtypes.bfloat16) for k, v in c.items()}


CDEFS = [("TRIZA", [NROW, 128]), ("TRIZB", [NROW, 128]),
         ("TRIZBA", [NROW, 128]),
         ("CBA0", [NROW, 32]), ("CBA1", [NROW, 32]), ("CBA2", [NROW, 32]),
         ("CBB0", [NROW, 32]), ("CBB1", [NROW, 32]), ("CBB2", [NROW, 32]),
         ("XPB", [NROW, 32])]


def _build_kernel(num_devices, nreps=1):
    nc = bacc.Bacc("TRN2", debug=False, num_devices=num_devices,
                   target_bir_lowering=False)

    # Ln+Exp live together in "natural_log_exp_and_others"; pin that one
    # table for the whole program to avoid dynamic table reloads.
    from concourse.hw_specs import get_activation_tables
    import concourse.bacc as _bacc_mod

    def _pinned_act_table_loads():
        tables = []
        for name, funcs in get_activation_tables(nc.m.arch).items():
            tables.append((name, funcs if name == "natural_log_exp_and_others"
                           else set()))
        _bacc_mod._bass_rust.insert_act_table_loads(nc, tables)

    nc.insert_act_table_loads = _pinned_act_table_loads
    tc = tile.TileContext(nc)

    dwTA = nc.dram_tensor("dwTA", [128, NCOL], BF16, kind="ExternalInput")
    dwTB = nc.dram_tensor("dwTB", [128, NCOL], BF16, kind="ExternalInput")
    cins = {n: nc.dram_tensor(n, s, BF16, kind="ExternalInput")
            for n, s in CDEFS}
    lbin = nc.dram_tensor("LB", [128, 1], F32, kind="ExternalInput")
    Zout = nc.dram_tensor("Zout", [NQ * 128, 512], F32, kind="ExternalOutput")

    NT = 2 * NQ                     # quad-slabs per pass (A/B interleaved)

    with tc:
        with tc.tile_pool(name="consts", bufs=1) as cpool, \
             tc.tile_pool(name="inp", bufs=4) as ipool, \
             tc.tile_pool(name="lgp", bufs=3) as lpool, \
             tc.tile_pool(name="xwp", bufs=LQ + 2) as xpool, \
             tc.tile_pool(name="p1p", bufs=2) as ppool, \
             tc.tile_pool(name="stg", bufs=2) as spool, \
             tc.tile_pool(name="ps_pref", bufs=3, space="PSUM") as p_pref, \
             tc.tile_pool(name="ps_y", bufs=2, space="PSUM") as p_y:

            C = {}
            for n, s in CDEFS:
                C[n] = cpool.tile(s, BF16, name=f"c_{n}", tag=f"c_{n}")
                nc.sync.dma_start(C[n][:], cins[n].ap())
            LB = cpool.tile([128, 1], F32, name="c_LB", tag="c_LB")
            nc.sync.dma_start(LB[:], lbin.ap())

            dwt = {}
            lg = {}
            xq = {}
            wq = {}
            p2q = {}
            ypt = {}

            def dma_in(j):
                dwt[j] = ipool.tile([128, 4096], BF16, name="dw", tag="dw")
                cols = slice(2048 * j, 2048 * (j + 1))
                nc.sync.dma_start(dwt[j][:, 0:2048], dwTA.ap()[:, cols])
                nc.sync.dma_start(dwt[j][:, 2048:4096], dwTB.ap()[:, cols])

            def ln_op(t):
                j, ph = t // 2, t % 2
                if ph == 0:
                    lg[j] = lpool.tile([128, 4096], BF16, name="lg", tag="lg")
                so = 2048 * ph
                nc.scalar.activation(lg[j][:, so:so + 2048],
                                     dwt[j][:, so:so + 2048], AF.Ln,
                                     bias=LB[:], scale=SIGMA)

            def front(t):
                j, ph = t // 2, t % 2
                so = 2048 * ph
                if t + 1 < NT:
                    ln_op(t + 1)
                TRZ = C["TRIZB"] if ph else C["TRIZA"]
                x = xpool.tile([128, 2048], BF16, name="x", tag="x")
                for h in range(2):
                    pref = p_pref.tile([128, 1024], F32, name="pref",
                                       tag="pref")
                    for k2 in range(2):
                        c0 = 1024 * h + 512 * k2
                        oc = slice(512 * k2, 512 * (k2 + 1))
                        nc.tensor.matmul(pref[:, oc], TRZ[:],
                                         lg[j][0:NROW, so + c0:so + c0 + 512],
                                         start=True, stop=(ph == 0))
                        if ph == 1:
                            nc.tensor.matmul(pref[:, oc], C["TRIZBA"][:],
                                             lg[j][0:NROW, c0:c0 + 512],
                                             start=False, stop=True)
                    nc.scalar.activation(x[:, 1024 * h:1024 * (h + 1)],
                                         pref[:], AF.Exp)
                dws = dwt[j][:, so:so + 2048]
                w = xpool.tile([128, 2048], BF16, name="w", tag="w")
                nc.vector.tensor_tensor(w[:], x[:], dws, op=ALU.mult)
                p1 = ppool.tile([128, 2048], BF16, name="p1", tag="p1")
                nc.vector.tensor_tensor(p1[:], w[:], x[:], op=ALU.mult)
                p2 = xpool.tile([128, 2048], BF16, name="p2", tag="p2")
                nc.vector.tensor_tensor(p2[:], p1[:], x[:], op=ALU.mult)
                xq[t], wq[t], p2q[t] = x, w, p2

            def contract(t):
                j, ph = t // 2, t % 2
                so = 2048 * ph
                if ph == 0:
                    ypt[j] = p_y.tile([128, 512], F32, name="yp", tag="yp")
                yp = ypt[j]
                cb = ("CBB0", "CBB1", "CBB2") if ph else \
                     ("CBA0", "CBA1", "CBA2")
                for jj in range(4):
                    lc = slice(512 * jj, 512 * (jj + 1))
                    ys = yp[32 * jj:32 * (jj + 1), :]
                    tp = (0, 32 * jj)
                    nc.tensor.matmul(ys, C[cb[0]][:],
                                     dwt[j][0:NROW, so + 512 * jj:
                                            so + 512 * (jj + 1)],
                                     start=(ph == 0), stop=False,
                                     tile_position=tp)
                    nc.tensor.matmul(ys, C[cb[1]][:], wq[t][0:NROW, lc],
                                     start=False, stop=False,
                                     tile_position=tp)
                    nc.tensor.matmul(ys, C[cb[2]][:], p2q[t][0:NROW, lc],
                                     start=False, stop=False,
                                     tile_position=tp)
                    if ph == 1:
                        nc.tensor.matmul(ys, C["XPB"][:],
                                         xq[t][0:NROW, lc],
                                         start=False, stop=True,
                                         tile_position=tp)
                del xq[t], wq[t], p2q[t]
                if ph == 1:
                    del dwt[j], lg[j]
                    stg = spool.tile([128, 512], F32, name="st", tag="st")
                    nc.vector.tensor_copy(stg[:], yp[:])
                    nc.sync.dma_start(Zout.ap()[128 * j:128 * (j + 1), :],
                                      stg[:])

            for rep in range(nreps):
                dma_in(0)
                dma_in(1)
                ln_op(0)
                for t in range(NT + LQ):
                    if t < NT:
                        j, ph = t // 2, t % 2
                        if ph == 0 and j + 2 < NQ:
                            dma_in(j + 2)
                        front(t)
                    if t >= LQ:
                        contract(t - LQ)

    nc.compile()
    return nc


_CACHE = {}
_LAST_IN_MAPS = None


def kernel(dw, t_grid, W1, b1, W2, b2, W3, b3, Y0):
    import ml_dtypes
    dw = np.asarray(dw, np.float32)
    t_grid = np.asarray(t_grid, np.float32)
    B = dw.shape[0]
    assert B == B_FULL and dw.shape[1] == M
    a50y0 = np.float32(A ** M * np.float32(Y0))

    if "nc" not in _CACHE:
        _CACHE["nc"] = _build_kernel(NCORES)
    nc = _CACHE["nc"]

    consts = _build_consts(np.asarray(W1, np.float32),
                           np.asarray(b1, np.float32),
                           np.asarray(W2, np.float32),
                           np.asarray(b2, np.float32),
                           np.asarray(W3, np.float32),
                           np.asarray(b3, np.float32), t_grid[0])

    dwb = dw.astype(ml_dtypes.bfloat16)
    in_maps = []
    for ci in range(NCORES):
        blk = dwb[ci * B_CORE:(ci + 1) * B_CORE]
        pad = np.zeros((G * NCOL, M), ml_dtypes.bfloat16)
        pad[:B_CORE] = blk
        v = pad.reshape(G, NCOL, M)
        dwTA = np.zeros((128, NCOL), ml_dtypes.bfloat16)
        dwTB = np.zeros((128, NCOL), ml_dtypes.bfloat16)
        dwTA[0:NROW] = v[:, :, 0:SH].transpose(0, 2, 1).reshape(NROW, NCOL)
        dwTB[0:NROW] = v[:, :, SH:M].transpose(0, 2, 1).reshape(NROW, NCOL)
        mci = dict(consts)
        mci["dwTA"] = dwTA
        mci["dwTB"] = dwTB
        mci["LB"] = np.full((128, 1), A, np.float32)
        in_maps.append(mci)

    global _LAST_IN_MAPS
    _LAST_IN_MAPS = in_maps
    res = bass_utils.run_bass_kernel_spmd(nc, in_maps,
                                          core_ids=list(range(NCORES)))

    Y = np.empty((B_FULL,), np.float32)
    S = np.empty((B_FULL,), np.float32)
    for ci in range(NCORES):
        # Zout row 128j + 32jj + q: q in 0..4 -> Y of group q, 5..9 -> S50;
        # column c of that row -> path (q%5)*NCOL + 2048j + 512jj + c
        Z = res.results[ci]["Zout"].reshape(NQ, 4, 32, 512)
        Ymap = Z[:, :, 0:G, :].transpose(2, 0, 1, 3).reshape(G * NCOL)
        Smap = Z[:, :, G:2 * G, :].transpose(2, 0, 1, 3).reshape(G * NCOL)
        Y[ci * B_CORE:(ci + 1) * B_CORE] = Ymap[:B_CORE]
        S[ci * B_CORE:(ci + 1) * B_CORE] = Smap[:B_CORE]
    Y += a50y0
    return Y[:, None], S[:, None]
